# revision 1
# baseline (speedup 1.0000x reference)
"""Trainium2 Bass kernel for the CellLoss problem.

loss = mean_i [ 1/(x[i, l_i] + 0.1) + sum_j x[i,j] * (x[i,j] > x[i, l_i]) ]
with x: [131072, 256] f32, l: [131072] int labels in [0, 256).

Pure data parallel across 8 NeuronCores (16384 rows each). Per core,
partition p owns rows [p*128, (p+1)*128) of the shard; tile t is the
[128, 256] block of row p*128+t per partition.

Per tile:
  gather (DVE): g[p] = sum_j (iota==l_p)*x via one fused
      scalar_tensor_tensor (stt) with a per-row sum accumulator.
  margin, two engine variants cycled by PATTERN:
   "D": one more DVE stt, (x is_gt g) mult x with row-sum accumulator.
   "A": scalar-engine Relu(x-g) and Sign(x-g) passes writing bf16 tiles;
      the idle tensor engine then accumulates the GLOBAL sums in PSUM:
      ones^T @ relu-pairs, and (-g)^T @ sign per tile. Using
      sum_i g_i*cnt_i = (sum g*sign + 255*sum g)/2 (sign(0)=0 at the
      label), the margin needs only these global sums.
Tail: inv = 1/(g+0.1); per-row totals + the A-tile 127.5*g correction;
partition sum via ones-matmul; one f32 partial per core; the host sums
the 8 partials and divides by B.

bf16 is used ONLY for relu magnitudes (unbiased rounding, ~1e-6 effect)
and the exact-representable sign/one-weights; g itself stays exact f32
everywhere that matters (inv term, compares); the bf16 -g weight only
scales the count term (~1e-5 effect).

This walrus accepts one sync wait per instruction; Tile can emit
several. _split_multi_waits() hoists extras onto Drain carriers.
"""

import numpy as np
from contextlib import ExitStack

import concourse.bass as bass
import concourse.mybir as mybir
import concourse.tile as tile
from concourse.bass_utils import run_bass_kernel_spmd

F32 = mybir.dt.float32
BF16 = mybir.dt.bfloat16

B, C = 131072, 256
N_CORES = 8
B_LOCAL = B // N_CORES          # 16384
P = 128
N_TILES = B_LOCAL // P          # 128
TILES_PER_DMA = 16              # [128, 4096] f32 = 2 MiB per DMA
N_CHUNKS = N_TILES // TILES_PER_DMA

# margin engine per tile, cyclic ("D" DVE / "A" scalar engine);
# cycle length must divide 128; "A" tiles must form the cycle tail and
# their count per cycle must be even (they pair up for the relu matmul)
PATTERN = list("DDDDAAAAAAAAAAAA")

_NC_CACHE = {}
LAST_RESULTS = None
SPLIT_WAITS = True   # off for CoreSim (its event loop rejects bare Drains)
TRACE = False
TRACE_KW = {}


def _split_multi_waits(nc):
    for f in nc.m.functions:
        for blk in f.blocks:
            insts = list(blk.instructions)
            out = []
            changed = False
            for inst in insts:
                si = inst.sync_info
                if si is not None and si.on_wait is not None and len(si.on_wait) > 1:
                    waits = list(si.on_wait)
                    for w in waits[:-1]:
                        d = mybir.InstDrain(
                            name=nc.get_next_instruction_name(),
                            ins=[], outs=[], bass_is_fusable=False)
                        d.engine = inst.engine
                        d.sync_info = mybir.SyncInfo(on_wait=[w], on_update=[])
                        out.append(d)
                    inst.sync_info = mybir.SyncInfo(
                        on_wait=[waits[-1]], on_update=list(si.on_update or []))
                    changed = True
                out.append(inst)
            if changed:
                blk.instructions = out


def _assignment():
    assert N_TILES % len(PATTERN) == 0
    return [PATTERN[t % len(PATTERN)] for t in range(N_TILES)]


def build_nc():
    key = (tuple(_assignment()), SPLIT_WAITS)
    if key in _NC_CACHE:
        return _NC_CACHE[key]

    assign = _assignment()
    a_tiles = [t for t, c in enumerate(assign) if c == "A"]
    acol = {t: i for i, t in enumerate(a_tiles)}
    n_a = len(a_tiles)
    assert n_a % 2 == 0

    nc = bass.Bass()
    x = nc.declare_dram_parameter("x", [B_LOCAL, C], F32, isOutput=False)
    lbl = nc.declare_dram_parameter("lbl", [P, N_TILES], F32, isOutput=False)
    out = nc.declare_dram_parameter("out", [1, 1], F32, isOutput=True)

    xv = x.rearrange("(p t) c -> p (t c)", p=P, t=N_TILES)

    with tile.TileContext(nc) as tc, ExitStack() as ctx:
        singles = ctx.enter_context(tc.tile_pool(name="singles", bufs=1))
        xpool = ctx.enter_context(tc.tile_pool(name="x", bufs=3))
        scr = ctx.enter_context(tc.tile_pool(name="scr", bufs=4))
        prs = ctx.enter_context(tc.tile_pool(name="prs", bufs=4))
        psum = ctx.enter_context(tc.tile_pool(name="psum", bufs=1, space="PSUM"))

        lbl_sb = singles.tile([P, N_TILES], F32)
        nc.sync.dma_start(lbl_sb[:], lbl[:])

        iota_i = singles.tile([P, C], mybir.dt.int32)
        nc.gpsimd.iota(iota_i[:], pattern=[[1, C]], base=0, channel_multiplier=0)
        iota_f = singles.tile([P, C], F32)
        nc.vector.tensor_copy(iota_f[:], iota_i[:])

        ones = singles.tile([P, 1], F32)
        nc.vector.memset(ones[:], 1.0)

        G = singles.tile([P, N_TILES], F32)
        M = singles.tile([P, N_TILES], F32)      # D-tile margins; A cols = 0
        if n_a:
            ones_bf = singles.tile([P, 1], BF16)
            nc.vector.memset(ones_bf[:], 1.0)
            nc.vector.memset(M[:], 0.0)
            NGC = singles.tile([P, n_a], F32)    # -g (f32, ACT bias)
            ps_r = psum.tile([P, 512], F32, tag="ps_r")
            ps_s = [psum.tile([P, 512], F32, tag=f"ps_s{i}", name=f"ps_s{i}")
                    for i in range(2)]

        mm_r = 0
        mm_s = [0, 0]
        a_seen = 0
        rpair = None
        for chunk in range(N_CHUNKS):
            xw = xpool.tile([P, TILES_PER_DMA * C], F32, name="xw")
            nc.sync.dma_start(
                xw[:],
                xv[:, chunk * TILES_PER_DMA * C:(chunk + 1) * TILES_PER_DMA * C])
            for kk in range(TILES_PER_DMA):
                t = chunk * TILES_PER_DMA + kk
                xb = xw[:, kk * C:(kk + 1) * C]
                lc = lbl_sb[:, t:t + 1]
                gc = G[:, t:t + 1]
                sel = scr.tile([P, C], F32, tag="sel", name="sel")
                nc.vector.scalar_tensor_tensor(
                    out=sel[:], in0=iota_f[:], scalar=lc, in1=xb,
                    op0=mybir.AluOpType.is_equal, op1=mybir.AluOpType.mult,
                    accum_out=gc)
                if assign[t] == "D":
                    mp = scr.tile([P, C], F32, tag="mp", name="mp")
                    nc.vector.scalar_tensor_tensor(
                        out=mp[:], in0=xb, scalar=gc, in1=xb,
                        op0=mybir.AluOpType.is_gt, op1=mybir.AluOpType.mult,
                        accum_out=M[:, t:t + 1])
                else:  # "A"
                    j = acol[t]
                    ng = NGC[:, j:j + 1]
                    nc.vector.tensor_scalar_mul(ng, gc, -1.0)
                    u = a_seen % 2
                    if u == 0:
                        rpair = prs.tile([P, 2 * C], BF16, tag="rpair",
                                         name="rpair")
                    rb = rpair[:, u * C:(u + 1) * C]
                    nc.scalar.activation(
                        rb, xb, mybir.ActivationFunctionType.Relu,
                        bias=ng, scale=1.0)
                    # sign(g - x) = -sign(x - g): bias is the raw G column,
                    # no negate needed; g^T @ sign(g-x) equals the
                    # (-g)^T @ sign(x-g) the tail expects.
                    sg = scr.tile([P, C], F32, tag="sg", name="sg")
                    nc.scalar.activation(
                        sg[:], xb, mybir.ActivationFunctionType.Sign,
                        bias=gc, scale=-1.0)
                    nc.tensor.matmul(ps_s[u][:1, :C], gc, sg[:],
                                     start=(mm_s[u] == 0),
                                     stop=(mm_s[u] == n_a // 2 - 1))
                    mm_s[u] += 1
                    if u == 1:
                        nc.tensor.matmul(ps_r[:1, :], ones_bf[:], rpair[:],
                                         start=(mm_r == 0),
                                         stop=(mm_r == n_a // 2 - 1))
                        mm_r += 1
                    a_seen += 1

        # ---- tail ------------------------------------------------------
        tmp = scr.tile([P, N_TILES], F32, tag="tail", name="tmp")
        nc.vector.tensor_scalar_add(tmp[:], G[:], 0.1)
        inv = scr.tile([P, N_TILES], F32, tag="tail2", name="inv")
        nc.vector.reciprocal(inv[:], tmp[:])
        tot = scr.tile([P, N_TILES], F32, tag="tail3", name="tot")
        nc.vector.tensor_tensor(out=tot[:], in0=inv[:], in1=M[:],
                                op=mybir.AluOpType.add)
        rows = singles.tile([P, 1], F32)
        nc.vector.tensor_reduce(rows[:], tot[:], axis=mybir.AxisListType.X,
                                op=mybir.AluOpType.add)
        if n_a:
            L = len(PATTERN)
            nA = sum(1 for c in PATTERN if c == "A")
            a0 = L - nA
            assert all(c == "A" for c in PATTERN[a0:])
            g_a = G.rearrange("p (u k) -> p u k", k=L)[:, :, a0:]
            rows_ga = singles.tile([P, 1], F32)
            nc.vector.tensor_reduce(rows_ga[:], g_a,
                                    axis=mybir.AxisListType.XY,
                                    op=mybir.AluOpType.add)
            rows2 = singles.tile([P, 1], F32)
            nc.vector.tensor_scalar(out=rows2[:], in0=rows_ga[:],
                                    scalar1=127.5, scalar2=None,
                                    op0=mybir.AluOpType.mult)
            rows3 = singles.tile([P, 1], F32)
            nc.vector.tensor_tensor(out=rows3[:], in0=rows[:], in1=rows2[:],
                                    op=mybir.AluOpType.add)
            rows = rows3

        ps_fin = psum.tile([P, 8], F32, tag="fin")
        nc.tensor.matmul(ps_fin[:1, :1], ones[:], rows[:])

        fin = singles.tile([1, 1], F32)
        nc.vector.tensor_copy(fin[:], ps_fin[:1, :1])
        acc_terms = [fin]
        if n_a:
            # + sum(ps_r) - 0.5*sum(ps_s0 + ps_s1)
            cb = singles.tile([1, 1024], F32)
            nc.vector.tensor_copy(cb[:, 0:512], ps_r[:1, :])
            nc.vector.tensor_copy(cb[:, 512:768], ps_s[0][:1, :C])
            nc.vector.tensor_copy(cb[:, 768:1024], ps_s[1][:1, :C])
            tot1 = singles.tile([1, 1], F32)
            nc.vector.tensor_reduce(tot1[:], cb[:, 0:512],
                                    axis=mybir.AxisListType.X,
                                    op=mybir.AluOpType.add)
            # ps_s carries (-g)*sign sums; margin needs +(g*sign)/2
            sc = singles.tile([1, 512], F32)
            nc.vector.tensor_scalar(out=sc[:], in0=cb[:, 512:1024],
                                    scalar1=-0.5, scalar2=None,
                                    op0=mybir.AluOpType.mult)
            tot2 = singles.tile([1, 1], F32)
            nc.vector.tensor_reduce(tot2[:], sc[:],
                                    axis=mybir.AxisListType.X,
                                    op=mybir.AluOpType.add)
            acc_terms += [tot1, tot2]
        res = acc_terms[0]
        for ti, term in enumerate(acc_terms[1:]):
            nxt = singles.tile([1, 1], F32, name=f"sumchain{ti}")
            nc.vector.tensor_tensor(out=nxt[:], in0=res[:], in1=term[:],
                                    op=mybir.AluOpType.add)
            res = nxt
        nc.sync.dma_start(out[:], res[:])

    if SPLIT_WAITS:
        _split_multi_waits(nc)
    _NC_CACHE[key] = nc
    return nc


def _prep_inputs(rna_cell_out, rna_cell_label):
    x = np.ascontiguousarray(np.asarray(rna_cell_out, dtype=np.float32))
    l = np.asarray(rna_cell_label).astype(np.int64)
    assert x.shape == (B, C) and l.shape == (B,)
    in_maps = []
    for i in range(N_CORES):
        xs = x[i * B_LOCAL:(i + 1) * B_LOCAL]
        ls = l[i * B_LOCAL:(i + 1) * B_LOCAL]
        lbl = ls.reshape(P, N_TILES).astype(np.float32)
        in_maps.append({"x": xs, "lbl": np.ascontiguousarray(lbl)})
    return in_maps


def kernel(rna_cell_out, rna_cell_label):
    global LAST_RESULTS
    nc = build_nc()
    in_maps = _prep_inputs(rna_cell_out, rna_cell_label)
    res = run_bass_kernel_spmd(nc, in_maps, list(range(N_CORES)),
                               trace=TRACE, **TRACE_KW)
    LAST_RESULTS = res
    parts = [float(res.results[i]["out"][0, 0]) for i in range(N_CORES)]
    loss = np.float32(np.sum(np.array(parts, dtype=np.float64)) / B)
    return np.array([loss], dtype=np.float32)

